# revision 6
# baseline (speedup 1.0000x reference)
"""Chamfer distance loss kernel v2: single-emission + row-min tree.

Changes vs baseline (dual emission):
  - Each i-tile is emitted ONCE: M[i in tile t, all j] (side 0 orientation
    only). PE work and ACT drain volume are halved.
  - Per-source-point mins (colmins): unchanged path — elementwise bf16 min
    accumulation across tiles (DVE 2x mode) + PE-transpose partition reduce.
  - Per-template-point mins (rowmins): computed per tile directly from the
    drained bf16 stage with a TT-min halving tree (DVE 2x) + one small
    free-axis reduce -> [128,1] per tile. No second orientation, no second
    accumulator, no second transpose pass.
  Predicted engine busy per core: ACT ~255us (drains), DVE ~325us
  (acc 140 + trees 170 + transpose-reduce 10), PE ~56us. DVE-bound ~335us
  vs baseline's ACT-bound ~460us.
"""

import numpy as np

M_BATCH = 16
N = 4096
D = 3
N_CORES = 8
NB = M_BATCH // N_CORES  # batches per core
P = 128
IT = N // P  # 32 tiles per side
K_AUG = 15

# in-kernel repetition count (measurement only; 1 for production)
LOOP_REPS = 1

_CACHE = {}


def _build_nc():
    import concourse.bacc as bacc
    import concourse.tile as tile
    from concourse import mybir
    from concourse.masks import make_identity
    from contextlib import ExitStack, nullcontext

    F32 = mybir.dt.float32
    BF16 = mybir.dt.bfloat16
    FP16 = mybir.dt.float16
    X = mybir.AxisListType.X
    MIN = mybir.AluOpType.min

    JW = 1024  # psum tile width (2 banks)
    JC = N // JW  # 4 psum tiles per emitted tile

    nc = bacc.Bacc("TRN2", target_bir_lowering=False)
    lhsT_d = nc.declare_dram_parameter("lhsT", [NB, K_AUG, N], FP16, isOutput=False)
    rhs_d = nc.declare_dram_parameter("rhs", [NB, K_AUG, N], FP16, isOutput=False)
    mins_d = nc.declare_dram_parameter("mins", [NB, 2, P, IT], F32, isOutput=True)

    with ExitStack() as ctx:
        tc = ctx.enter_context(tile.TileContext(nc))
        consts = ctx.enter_context(tc.tile_pool(name="consts", bufs=1))
        inputs = ctx.enter_context(tc.tile_pool(name="inputs", bufs=2))
        # bufs=6/6 was tried and measured 328us (worse) — keep 4/3.
        stages = ctx.enter_context(tc.tile_pool(name="stages", bufs=4))
        rpool = ctx.enter_context(tc.tile_pool(name="rpool", bufs=3))
        accs = ctx.enter_context(tc.tile_pool(name="accs", bufs=2))
        outs = ctx.enter_context(tc.tile_pool(name="outs", bufs=2))
        psum = ctx.enter_context(tc.tile_pool(name="psum", bufs=3, space="PSUM"))
        tpsum = ctx.enter_context(tc.tile_pool(name="tpsum", bufs=2, space="PSUM"))

        ident = consts.tile([P, P], BF16)
        make_identity(nc, ident)

        loop_ctx = tc.For_i(0, LOOP_REPS, 1) if LOOP_REPS > 1 else nullcontext()
        with loop_ctx:
          for b in range(NB):
            lhsT_s = inputs.tile([K_AUG, N], FP16, tag="lhsT")
            rhs_s = inputs.tile([K_AUG, N], FP16, tag="rhs")
            nc.sync.dma_start(out=lhsT_s, in_=lhsT_d[b])
            nc.sync.dma_start(out=rhs_s, in_=rhs_d[b])

            # accB[:, j] accumulates min over i-tiles of M[i, j]
            accB = accs.tile([P, N], BF16, tag="accB")
            colmins = outs.tile([P, IT], F32, tag="colmins")
            rowmins = outs.tile([P, IT], F32, tag="rowmins")

            for t in range(IT):
                stage = stages.tile([P, N], BF16, tag="stage")
                for q in range(JC):
                    ps = psum.tile([P, JW], F32, tag="mm")
                    for h in range(JW // 512):
                        nc.tensor.matmul(
                            ps[:, h * 512 : (h + 1) * 512],
                            lhsT_s[:, t * P : (t + 1) * P],
                            rhs_s[:, q * JW + h * 512 : q * JW + (h + 1) * 512],
                            start=True,
                            stop=True,
                        )
                    # drain PSUM -> SBUF bf16 (all on ACT; DVE is the
                    # bottleneck in this structure)
                    nc.scalar.copy(out=stage[:, q * JW : (q + 1) * JW], in_=ps)
                # col-min accumulate (DVE 2x bf16)
                if t == 0:
                    nc.vector.tensor_copy(out=accB, in_=stage)
                else:
                    nc.vector.tensor_tensor(accB, stage, accB, MIN)
                # row-min tree: 4096 -> 2048 -> 1024 -> 512 -> [P,1]
                r1 = rpool.tile([P, N // 2], BF16, tag="r1")
                nc.vector.tensor_tensor(r1, stage[:, : N // 2], stage[:, N // 2 :], MIN)
                r2 = rpool.tile([P, N // 4], BF16, tag="r2")
                nc.vector.tensor_tensor(r2, r1[:, : N // 4], r1[:, N // 4 :], MIN)
                r3 = rpool.tile([P, N // 8], BF16, tag="r3")
                nc.vector.tensor_tensor(r3, r2[:, : N // 8], r2[:, N // 8 :], MIN)
                nc.vector.tensor_reduce(
                    out=rowmins[:, t : t + 1], in_=r3, axis=X, op=MIN
                )

            # partition-axis min of the col accumulator: PE-transpose 128x128
            # blocks, 8 at a time into one PSUM bank, one 3D reduce per group.
            for c8 in range(IT // 8):
                tp = tpsum.tile([P, 8, P], BF16, tag="tp")
                for k in range(8):
                    nc.tensor.transpose(
                        tp[:, k, :],
                        accB[:, (c8 * 8 + k) * P : (c8 * 8 + k + 1) * P],
                        ident,
                    )
                nc.vector.tensor_reduce(
                    out=colmins[:, c8 * 8 : (c8 + 1) * 8], in_=tp, axis=X, op=MIN
                )

            nc.sync.dma_start(out=mins_d[b, 0], in_=colmins)
            nc.sync.dma_start(out=mins_d[b, 1], in_=rowmins)

    nc.compile()
    return nc


def _get_nc():
    if "nc" not in _CACHE:
        _CACHE["nc"] = _build_nc()
    return _CACHE["nc"]


def _prep_inputs(template, source):
    """Build split-fp16 augmented [m, 15, n] operands (same as baseline)."""
    t = np.ascontiguousarray(template, dtype=np.float32)
    s = np.ascontiguousarray(source, dtype=np.float32)

    f16 = np.float16

    def split2(x):
        h = x.astype(f16).astype(np.float32)
        l = (x - h).astype(f16).astype(np.float32)
        return h, l

    def split3(x):
        h = x.astype(f16).astype(np.float32)
        r = x - h
        m = r.astype(f16).astype(np.float32)
        l = (r - m).astype(f16).astype(np.float32)
        return h, m, l

    ah, al = split2(t)
    bh, bl = split2(s)
    a2 = (t.astype(np.float64) ** 2).sum(-1).astype(np.float32)
    b2 = (s.astype(np.float64) ** 2).sum(-1).astype(np.float32)
    a2h, a2m, a2l = split3(a2)
    b2h, b2m, b2l = split3(b2)
    ones = np.ones_like(a2)

    lrows = []
    rrows = []
    for c in range(3):
        lrows += [-2.0 * ah[..., c], (-2.0 / 32.0) * ah[..., c], -128.0 * al[..., c]]
        rrows += [bh[..., c], 32.0 * bl[..., c], bh[..., c] / 64.0]
    lrows += [a2h, 32.0 * a2m, 2048.0 * a2l, ones, ones / 32.0, ones / 2048.0]
    rrows += [ones, ones / 32.0, ones / 2048.0, b2h, 32.0 * b2m, 2048.0 * b2l]

    lhsT = np.stack(lrows, axis=1).astype(f16)
    rhs = np.stack(rrows, axis=1).astype(f16)
    return np.ascontiguousarray(lhsT), np.ascontiguousarray(rhs)


def run(template, source, trace=False):
    """Returns (result_scalar, exec_time_ns_or_None)."""
    from concourse import bass_utils

    nc = _get_nc()
    lhsT, rhs = _prep_inputs(template, source)
    in_maps = [
        {
            "lhsT": np.ascontiguousarray(lhsT[c * NB : (c + 1) * NB]),
            "rhs": np.ascontiguousarray(rhs[c * NB : (c + 1) * NB]),
        }
        for c in range(N_CORES)
    ]
    res = bass_utils.run_bass_kernel_spmd(
        nc, in_maps, core_ids=list(range(N_CORES)), trace=trace
    )
    mins = np.stack([r["mins"] for r in res.results])  # [8, NB, 2, P, IT]
    total = np.sqrt(np.maximum(mins.astype(np.float64), 0.0)).sum()
    out = np.float32(total / (2.0 * M_BATCH * N))
    return out, res.exec_time_ns


def kernel(template, source):
    out, _ = run(template, source, trace=False)
    return out
